# revision 12
# baseline (speedup 1.0000x reference)
"""Trainium2 Bass kernel for nn_Attention_78151224918608.

Dense transformer attention block: QKV proj + RoPE + GQA causal attention
+ output proj. Sharding: tensor-parallel over heads across 8 cores
(core c: Q heads 4c..4c+3, KV head c). Each core computes a partial
output (its heads through wo rows); host sums the 8 bf16 partials in
fp32 and casts to bf16.

Layout strategy (per core, per batch):
  - All matmul operands bf16; accumulation fp32 in PSUM.
  - Projections computed transposed: QKV^T[384, S] = wqkv^T @ x^T so that
    Q^T/K^T (head-dim on partitions) feed the scores matmul directly.
  - RoPE: even/odd pair interleave is folded into wq/wk/wo columns on the
    host (perm = evens-then-odds), turning the pair swap into a 32-row
    block swap done with a small permutation matmul on PE.
  - Scores computed transposed per (b,h): S^T[k,q] = K^T.T @ Q^T, so the
    softmax denominator and P@V both contract over k = partitions:
    PV lhsT = [V | ones-col] gives O^T rows 0:64 and sumexp in row 64.
  - Causal: only k-tiles <= q-tile are computed; diagonal 128x128 blocks
    get an additive triangular mask in PSUM before exp; fully-invalid
    column strips of the exp tile are memset to 0 afterwards.
  - exp on ScalarE reads PSUM strips [128, 1024] and writes bf16 SBUF.
  - Normalization: recip = 1/sumexp (DVE), broadcast across 64 partitions
    with a K=1 ones matmul, multiplied into O^T during evacuation.
"""

import sys

sys.path.insert(0, "/opt/trn_rl_repo")

import math
import numpy as np
import ml_dtypes

BF16 = ml_dtypes.bfloat16

# Problem constants (hardcoded per contract).
B = 2
S = 2048
D = 2048
N_HEADS = 32
N_KV_HEADS = 8
HD = 64
N_CORES = 8
HQ = N_HEADS // N_CORES  # 4 q heads per core
M_PROJ = HQ * HD + 2 * HD  # 384: [Q0 Q1 Q2 Q3 | K | V]
QTS = 512  # q tile size (free dim)
KTS = 128  # k tile size (partitions)
GRP = 2  # k-tiles per exp strip


def build_program(s=S, d=D, phase_log=None):
    import concourse.bass as bass
    import concourse.mybir as mybir
    import concourse.tile as tile
    from concourse import bacc

    def mark(label):
        if phase_log is not None:
            phase_log.append((label, len(nc.inst_map)))

    f32 = mybir.dt.float32
    bf16 = mybir.dt.bfloat16
    Exp = mybir.ActivationFunctionType.Exp
    Copy = mybir.ActivationFunctionType.Copy
    add_op = mybir.AluOpType.add
    mult_op = mybir.AluOpType.mult

    n_qt = s // QTS  # q tiles per batch
    n_dkt = d // 128  # contraction tiles for projections
    n_skt = s // KTS  # k tiles per batch
    n_nt = s // QTS  # token tiles (512) for proj free dim
    n_mo = (HQ * HD) // 128  # wo contraction tiles (2)

    nc = bacc.Bacc("TRN2", num_devices=N_CORES)
    xT_d = nc.declare_dram_parameter("xT", [B, d, s], bf16, isOutput=False)
    wqkv_d = nc.declare_dram_parameter("wqkv", [d, M_PROJ], bf16, isOutput=False)
    wo_d = nc.declare_dram_parameter("wo_s", [HQ * HD, d], bf16, isOutput=False)
    cos_d = nc.declare_dram_parameter("cosb", [128, s], bf16, isOutput=False)
    sin_d = nc.declare_dram_parameter("sinb", [128, s], bf16, isOutput=False)
    pmat_d = nc.declare_dram_parameter("pmat", [128, 128], bf16, isOutput=False)
    tri_d = nc.declare_dram_parameter("trimask", [128, 128], f32, isOutput=False)
    part_d = nc.declare_dram_parameter("part", [B * s, d], bf16, isOutput=True)

    with tile.TileContext(nc) as tc:
        with (
            tc.tile_pool(name="const", bufs=1) as cpool,
            tc.tile_pool(name="big", bufs=1) as bpool,
            tc.tile_pool(name="work", bufs=2) as wpool,
            tc.tile_pool(name="estrip", bufs=3) as epool,
            tc.tile_pool(name="outp", bufs=3) as opool,
            tc.tile_pool(name="psw", bufs=2, space="PSUM") as psw,
            tc.tile_pool(name="pssc", bufs=2, space="PSUM") as pssc,
            tc.tile_pool(name="psops", bufs=2, space="PSUM") as psops,
        ):
            # ---- constants / weights ----
            cos_sb = cpool.tile([128, s], bf16)
            sin_sb = cpool.tile([128, s], bf16)
            pmat_sb = cpool.tile([128, 128], bf16)
            tri_sb = cpool.tile([128, 128], f32)
            ones_sb = cpool.tile([1, 64], f32)
            wqkv_sb = cpool.tile([128, n_dkt, M_PROJ], bf16)
            wo_sb = cpool.tile([128, n_mo, d], bf16)

            nc.sync.dma_start(cos_sb[:], cos_d[:])
            nc.sync.dma_start(sin_sb[:], sin_d[:])
            nc.sync.dma_start(pmat_sb[:], pmat_d[:])
            nc.sync.dma_start(tri_sb[:], tri_d[:])
            nc.gpsimd.memset(ones_sb[:], 1.0)
            for kt in range(n_dkt):
                nc.sync.dma_start(
                    wqkv_sb[:, kt, :], wqkv_d[kt * 128 : (kt + 1) * 128, :]
                )
            for kt in range(n_mo):
                nc.sync.dma_start(wo_sb[:, kt, :], wo_d[kt * 128 : (kt + 1) * 128, :])

            # ---- per-batch persistent tiles ----
            tiles = {}

            def load_x(b):
                xT_sb = bpool.tile([128, n_dkt, s], bf16, tag="xT")
                tiles[("x", b)] = xT_sb
                for kt in range(n_dkt):
                    nc.sync.dma_start(
                        xT_sb[:, kt, :],
                        xT_d[b, kt * 128 : (kt + 1) * 128, :],
                    )

            def proj(b):
                mark(f"b{b}_proj")
                xT_sb = tiles[("x", b)]
                QT_sb = bpool.tile([128, n_mo, s], bf16, tag="QT")
                KT2_sb = bpool.tile([128, s], bf16, tag="KT2")
                VT_sb = bpool.tile([128, s], bf16, tag="VT")
                V_sb = bpool.tile([128, n_skt, 128], bf16, tag="V")
                tiles[("QT", b)] = QT_sb
                tiles[("KT2", b)] = KT2_sb
                tiles[("V", b)] = V_sb
                # ones column / zero pad for PV lhsT
                nc.gpsimd.memset(V_sb[:, :, 64:128], 0.0)
                nc.gpsimd.memset(V_sb[:, :, 64:65], 1.0)
                for m in (2, 0, 1):  # K/V first so attention can start early
                    for n in range(n_nt):
                        nsl = slice(n * QTS, (n + 1) * QTS)
                        ps = psw.tile([128, QTS], f32, tag="w")
                        for kt in range(n_dkt):
                            nc.tensor.matmul(
                                ps[:],
                                wqkv_sb[:, kt, m * 128 : (m + 1) * 128],
                                xT_sb[:, kt, nsl],
                                start=(kt == 0),
                                stop=(kt == n_dkt - 1),
                            )
                        if m < 2:
                            # two Q heads stacked: rope all 128 rows
                            q_raw = wpool.tile([128, QTS], bf16, tag="qraw")
                            nc.scalar.activation(q_raw[:], ps[:], Copy)
                            t1 = wpool.tile([128, QTS], bf16, tag="t1")
                            t2 = wpool.tile([128, QTS], bf16, tag="t2")
                            nc.vector.tensor_tensor(
                                t1[:], q_raw[:], cos_sb[:, nsl], mult_op
                            )
                            # t2 = swap(q_raw) * sin: partition-shifted reads
                            for r0, r1 in ((0, 32), (32, 0), (64, 96), (96, 64)):
                                nc.vector.tensor_tensor(
                                    t2[r0 : r0 + 32, :],
                                    q_raw[r1 : r1 + 32, :],
                                    sin_sb[r0 : r0 + 32, nsl],
                                    mult_op,
                                )
                            nc.vector.tensor_tensor(
                                QT_sb[:, m, nsl], t1[:], t2[:], add_op
                            )
                        else:
                            # rows 0:64 = K^T (rope), rows 64:128 = V^T (copy)
                            q_raw = wpool.tile([128, QTS], bf16, tag="qraw")
                            nc.scalar.activation(q_raw[0:64, :], ps[0:64, :], Copy)
                            t1 = wpool.tile([128, QTS], bf16, tag="t1")
                            t2 = wpool.tile([128, QTS], bf16, tag="t2")
                            nc.vector.tensor_tensor(
                                t1[0:64, :], q_raw[0:64, :], cos_sb[0:64, nsl], mult_op
                            )
                            for r0, r1 in ((0, 32), (32, 0)):
                                nc.vector.tensor_tensor(
                                    t2[r0 : r0 + 32, :],
                                    q_raw[r1 : r1 + 32, :],
                                    sin_sb[r0 : r0 + 32, nsl],
                                    mult_op,
                                )
                            nc.vector.tensor_tensor(
                                KT2_sb[0:64, nsl], t1[0:64, :], t2[0:64, :], add_op
                            )
                            # duplicate K^T into partitions 64:128 (row-group packing)
                            nc.vector.tensor_copy(
                                KT2_sb[64:128, nsl], KT2_sb[0:64, nsl]
                            )
                            # V^T: plain cast copy into partitions 64:128
                            nc.scalar.activation(
                                VT_sb[64:128, nsl], ps[64:128, :], Copy
                            )
                    if m == 2:
                        # V^T -> V (token-major) via DMA transpose
                        mark(f"b{b}_vtr")
                        for kt in range(n_skt):
                            nc.sync.dma_start_transpose(
                                V_sb[:, kt, 0:64],
                                VT_sb[64:128, kt * KTS : (kt + 1) * KTS],
                            )
                        mark(f"b{b}_proj2")

            def attn(b):
                mark(f"b{b}_attn")
                QT_sb = tiles[("QT", b)]
                KT2_sb = tiles[("KT2", b)]
                V_sb = tiles[("V", b)]
                OT_sb = bpool.tile([128, n_mo, s], bf16, tag="OT")
                tiles[("OT", b)] = OT_sb
                pending = []

                def normalize(hb2, m2, qsl2, ops2, rt2):
                    # recip already issued (DVE); broadcast + scale into OT
                    bps = psw.tile([128, QTS], f32, tag="w")
                    nc.tensor.matmul(
                        bps[0:64, :], ones_sb[:], rt2[:], start=True, stop=True
                    )
                    bsb = wpool.tile([64, QTS], f32, tag="bsb")
                    nc.any.tensor_copy(bsb[:], bps[0:64, :])
                    nc.vector.tensor_tensor(
                        OT_sb[hb2 : hb2 + 64, m2, qsl2],
                        ops2[0:64, :],
                        bsb[:],
                        mult_op,
                    )

                for h in range(HQ):
                    hb = (h % 2) * 64
                    qh = QT_sb[hb : hb + 64, h // 2, :]
                    kt2 = KT2_sb[hb : hb + 64, :]
                    for qt in range(n_qt):
                        qsl = slice(qt * QTS, (qt + 1) * QTS)
                        n_kt = (qt + 1) * (QTS // KTS)  # k tiles needed
                        ops = psops.tile([128, QTS], f32, tag="ops")
                        for g in range(0, n_kt, GRP):
                            kts = list(range(g, min(g + GRP, n_kt)))
                            sc = pssc.tile([128, GRP * QTS], f32, tag="sc")
                            e = epool.tile([128, GRP * QTS], bf16, tag="e")
                            for j, kt in enumerate(kts):
                                nc.tensor.matmul(
                                    sc[:, j * QTS : (j + 1) * QTS],
                                    kt2[:, kt * KTS : (kt + 1) * KTS],
                                    qh[:, qsl],
                                    start=True,
                                    stop=True,
                                )
                                o = kt * KTS - qt * QTS
                                if o >= 0:  # diagonal tile
                                    nc.vector.tensor_tensor(
                                        sc[:, j * QTS + o : j * QTS + o + 128],
                                        sc[:, j * QTS + o : j * QTS + o + 128],
                                        tri_sb[:],
                                        add_op,
                                    )
                            if g == 0 and pending:
                                # normalize the previous q-tile now; its recip
                                # had time to finish, so PE doesn't stall
                                normalize(*pending.pop())
                            nc.scalar.activation(
                                e[:, 0 : len(kts) * QTS], sc[:, 0 : len(kts) * QTS], Exp
                            )
                            for j, kt in enumerate(kts):
                                o = kt * KTS - qt * QTS
                                if o > 0:
                                    nc.gpsimd.memset(
                                        e[:, j * QTS : j * QTS + o], 0.0
                                    )
                                nc.tensor.matmul(
                                    ops[:],
                                    V_sb[:, kt, :],
                                    e[:, j * QTS : (j + 1) * QTS],
                                    start=(kt == 0),
                                    stop=(kt == n_kt - 1),
                                )
                        rt = wpool.tile([1, QTS], f32, tag="rt")
                        nc.vector.reciprocal(rt[:], ops[64:65, :])
                        pending.append((hb, h // 2, qsl, ops, rt))
                if pending:
                    normalize(*pending.pop())

            def wo_proj(b):
                mark(f"b{b}_wo")
                OT_sb = tiles[("OT", b)]
                for mt in range(s // 128):
                    msl = slice(mt * 128, (mt + 1) * 128)
                    osb = opool.tile([128, d], bf16, tag="osb")
                    for n in range(d // QTS):
                        nsl = slice(n * QTS, (n + 1) * QTS)
                        pool = psw if n % 2 == 0 else pssc
                        ps = pool.tile([128, QTS], f32, tag="w" if n % 2 == 0 else "sc")
                        for kt in range(n_mo):
                            nc.tensor.matmul(
                                ps[:],
                                OT_sb[:, kt, msl],
                                wo_sb[:, kt, nsl],
                                start=(kt == 0),
                                stop=(kt == n_mo - 1),
                            )
                        nc.any.tensor_copy(osb[:, nsl], ps[:])
                    nc.sync.dma_start(
                        part_d[b * s + mt * 128 : b * s + (mt + 1) * 128, :],
                        osb[:],
                    )

            load_x(0)
            proj(0)
            load_x(1)  # b1 input load overlaps b0 attention (SP order)
            attn(0)
            wo_proj(0)
            proj(1)
            attn(1)
            wo_proj(1)
    mark("end")
    nc.compile()
    return nc


# ---------------- host-side sharding ----------------

_PERM = np.concatenate([np.arange(0, HD, 2), np.arange(1, HD, 2)])  # evens, odds


def make_core_inputs(x, freqs_cos, freqs_sin, wq, wk, wv, wo, s=S, d=D):
    """Build per-core input maps (list of dicts, one per core)."""
    b = x.shape[0]
    xT = np.ascontiguousarray(np.transpose(x, (0, 2, 1))).astype(BF16)  # [B, D, S]

    cosT = np.ascontiguousarray(freqs_cos.T)  # [32, S]
    sinT = np.ascontiguousarray(freqs_sin.T)
    cosb = np.tile(np.concatenate([cosT, cosT], axis=0), (2, 1)).astype(BF16)  # [128,S]
    sinb = np.tile(
        np.concatenate([-sinT, sinT], axis=0), (2, 1)
    ).astype(BF16)

    pm = np.zeros((64, 64), np.float32)
    pm[np.arange(32), np.arange(32, 64)] = 1.0
    pm[np.arange(32, 64), np.arange(32)] = 1.0
    pmat = np.zeros((128, 128), np.float32)
    pmat[0:64, 0:64] = pm
    pmat[64:128, 64:128] = pm
    pmat = pmat.astype(BF16)

    p = np.arange(128)[:, None]
    f = np.arange(128)[None, :]
    trimask = np.where(f >= p, 0.0, -1e9).astype(np.float32)

    scale = 1.0 / math.sqrt(HD)
    in_maps = []
    for c in range(N_CORES):
        wq_c = np.concatenate(
            [
                wq[:, (4 * c + h) * HD : (4 * c + h + 1) * HD][:, _PERM]
                for h in range(HQ)
            ],
            axis=1,
        ) * scale
        wk_c = wk[:, c * HD : (c + 1) * HD][:, _PERM]
        wv_c = wv[:, c * HD : (c + 1) * HD]
        wqkv = np.concatenate([wq_c, wk_c, wv_c], axis=1).astype(BF16)  # [D, 384]
        wo_c = np.ascontiguousarray(
            wo[4 * c * HD : (4 * c + HQ) * HD, :]
        ).astype(BF16)  # [256, D] — O is in original d-order (V unpermuted)
        in_maps.append(
            {
                "xT": xT,
                "wqkv": wqkv,
                "wo_s": wo_c,
                "cosb": cosb,
                "sinb": sinb,
                "pmat": pmat,
                "trimask": trimask,
            }
        )
    return in_maps


_NC_CACHE = {}


def kernel(x, freqs_cos, freqs_sin, wq, wk, wv, wo):
    from concourse.bass_utils import run_bass_kernel_spmd

    x = np.asarray(x, np.float32)
    freqs_cos = np.asarray(freqs_cos, np.float32)
    freqs_sin = np.asarray(freqs_sin, np.float32)
    wq = np.asarray(wq, np.float32)
    wk = np.asarray(wk, np.float32)
    wv = np.asarray(wv, np.float32)
    wo = np.asarray(wo, np.float32)

    if "nc" not in _NC_CACHE:
        _NC_CACHE["nc"] = build_program()
    nc = _NC_CACHE["nc"]

    in_maps = make_core_inputs(x, freqs_cos, freqs_sin, wq, wk, wv, wo)
    res = run_bass_kernel_spmd(nc, in_maps, list(range(N_CORES)))
    acc = np.zeros((B * S, D), np.float32)
    for r in res.results:
        acc += np.asarray(r["part"], np.float32)
    return acc.reshape(B, S, D).astype(BF16)


# revision 17
# speedup vs baseline: 1.3259x; 1.3259x over previous
"""Trainium2 Bass kernel for nn_Attention_78151224918608.

Dense transformer attention block: QKV proj + RoPE + GQA causal attention
+ output proj. Sharding: tensor-parallel over heads across 8 cores
(core c: Q heads 4c..4c+3, KV head c). Each core computes a partial
output (its heads through wo rows); host sums the 8 bf16 partials in
fp32 and casts to bf16.

Layout strategy (per core, per batch):
  - All matmul operands bf16; accumulation fp32 in PSUM.
  - Projections computed transposed: QKV^T[384, S] = wqkv^T @ x^T so that
    Q^T/K^T (head-dim on partitions) feed the scores matmul directly.
  - RoPE: even/odd pair interleave is folded into wq/wk/wo columns on the
    host (perm = evens-then-odds), turning the pair swap into a 32-row
    block swap done with a small permutation matmul on PE.
  - Scores computed transposed per (b,h): S^T[k,q] = K^T.T @ Q^T, so the
    softmax denominator and P@V both contract over k = partitions:
    PV lhsT = [V | ones-col] gives O^T rows 0:64 and sumexp in row 64.
  - Causal: only k-tiles <= q-tile are computed; diagonal 128x128 blocks
    get an additive triangular mask in PSUM before exp; fully-invalid
    column strips of the exp tile are memset to 0 afterwards.
  - exp on ScalarE reads PSUM strips [128, 1024] and writes bf16 SBUF.
  - Normalization: recip = 1/sumexp (DVE), broadcast across 64 partitions
    with a K=1 ones matmul, multiplied into O^T during evacuation.
"""

import sys

sys.path.insert(0, "/opt/trn_rl_repo")

import math
import numpy as np
import ml_dtypes

BF16 = ml_dtypes.bfloat16

# Problem constants (hardcoded per contract).
B = 2
S = 2048
D = 2048
N_HEADS = 32
N_KV_HEADS = 8
HD = 64
N_CORES = 8
HQ = N_HEADS // N_CORES  # 4 q heads per core
M_PROJ = HQ * HD + 2 * HD  # 384: [Q0 Q1 Q2 Q3 | K | V]
QTS = 512  # q tile size (free dim)
KTS = 128  # k tile size (partitions)
GRP = 2  # k-tiles per exp strip


def build_program(s=S, d=D, phase_log=None):
    import concourse.bass as bass
    import concourse.mybir as mybir
    import concourse.tile as tile
    from concourse import bacc

    def mark(label):
        if phase_log is not None:
            phase_log.append((label, len(nc.inst_map)))

    f32 = mybir.dt.float32
    bf16 = mybir.dt.bfloat16
    Exp = mybir.ActivationFunctionType.Exp
    Copy = mybir.ActivationFunctionType.Copy
    add_op = mybir.AluOpType.add
    mult_op = mybir.AluOpType.mult

    n_qt = s // QTS  # q tiles per batch
    n_dkt = d // 128  # contraction tiles for projections
    n_skt = s // KTS  # k tiles per batch
    n_nt = s // QTS  # token tiles (512) for proj free dim
    n_mo = (HQ * HD) // 128  # wo contraction tiles (2)

    nc = bacc.Bacc("TRN2", num_devices=N_CORES)
    xT_d = nc.declare_dram_parameter("xT", [B, d, s], bf16, isOutput=False)
    wqkv_d = nc.declare_dram_parameter("wqkv", [d, M_PROJ], bf16, isOutput=False)
    wo_d = nc.declare_dram_parameter("wo_s", [HQ * HD, d], bf16, isOutput=False)
    cos_d = nc.declare_dram_parameter("cosb", [128, s], bf16, isOutput=False)
    sin_d = nc.declare_dram_parameter("sinb", [128, s], bf16, isOutput=False)
    tri_d = nc.declare_dram_parameter("trimask", [128, 128], f32, isOutput=False)
    part_d = nc.declare_dram_parameter("part", [B * s, d], bf16, isOutput=True)

    with tile.TileContext(nc) as tc:
        with (
            tc.tile_pool(name="const", bufs=1) as cpool,
            tc.tile_pool(name="big", bufs=1) as bpool,
            tc.tile_pool(name="work", bufs=2) as wpool,
            tc.tile_pool(name="estrip", bufs=3) as epool,
            tc.tile_pool(name="outp", bufs=3) as opool,
            tc.tile_pool(name="psw", bufs=2, space="PSUM") as psw,
            tc.tile_pool(name="pssc", bufs=2, space="PSUM") as pssc,
            tc.tile_pool(name="psops", bufs=2, space="PSUM") as psops,
        ):
            # ---- constants / weights ----
            cos_sb = cpool.tile([128, s], bf16)
            sin_sb = cpool.tile([128, s], bf16)
            tri_sb = cpool.tile([128, 128], f32)
            ones_sb = cpool.tile([1, 64], f32)
            wqkv_sb = cpool.tile([128, n_dkt, M_PROJ], bf16)
            wo_sb = cpool.tile([128, n_mo, d], bf16)

            nc.sync.dma_start(cos_sb[:], cos_d[:])
            nc.sync.dma_start(sin_sb[:], sin_d[:])
            nc.sync.dma_start(tri_sb[:], tri_d[:])
            nc.gpsimd.memset(ones_sb[:], 1.0)
            for kt in range(n_dkt):
                nc.sync.dma_start(
                    wqkv_sb[:, kt, :], wqkv_d[kt * 128 : (kt + 1) * 128, :]
                )
            for kt in range(n_mo):
                nc.sync.dma_start(wo_sb[:, kt, :], wo_d[kt * 128 : (kt + 1) * 128, :])

            # ---- per-batch persistent tiles ----
            tiles = {}

            def load_x(b):
                xT_sb = bpool.tile([128, n_dkt, s], bf16, tag="xT")
                tiles[("x", b)] = xT_sb
                for kt in range(n_dkt):
                    nc.sync.dma_start(
                        xT_sb[:, kt, :],
                        xT_d[b, kt * 128 : (kt + 1) * 128, :],
                    )

            def proj(b):
                mark(f"b{b}_proj")
                xT_sb = tiles[("x", b)]
                QT_sb = bpool.tile([128, n_mo, s], bf16, tag="QT")
                KT2_sb = bpool.tile([128, s], bf16, tag="KT2")
                VT_sb = bpool.tile([128, s], bf16, tag="VT")
                V_sb = bpool.tile([128, n_skt, 128], bf16, tag="V")
                tiles[("QT", b)] = QT_sb
                tiles[("KT2", b)] = KT2_sb
                tiles[("V", b)] = V_sb
                # ones column / zero pad for PV lhsT
                nc.gpsimd.memset(V_sb[:, :, 64:128], 0.0)
                nc.gpsimd.memset(V_sb[:, :, 64:65], 1.0)
                for m in (2, 0, 1):  # K/V first so attention can start early
                    for n in range(n_nt):
                        nsl = slice(n * QTS, (n + 1) * QTS)
                        ps = psw.tile([128, QTS], f32, tag="w")
                        for kt in range(n_dkt):
                            nc.tensor.matmul(
                                ps[:],
                                wqkv_sb[:, kt, m * 128 : (m + 1) * 128],
                                xT_sb[:, kt, nsl],
                                start=(kt == 0),
                                stop=(kt == n_dkt - 1),
                            )
                        if m < 2:
                            # two Q heads stacked: rope all 128 rows
                            q_raw = wpool.tile([128, QTS], bf16, tag="qraw")
                            nc.scalar.activation(q_raw[:], ps[:], Copy)
                            t1 = wpool.tile([128, QTS], bf16, tag="t1")
                            t2 = wpool.tile([128, QTS], bf16, tag="t2")
                            nc.vector.tensor_tensor(
                                t1[:], q_raw[:], cos_sb[:, nsl], mult_op
                            )
                            # swap(q_raw) via cross-base copies, then * sin
                            qsw = wpool.tile([128, QTS], bf16, tag="qsw")
                            for r0, r1 in ((0, 32), (32, 0), (64, 96), (96, 64)):
                                nc.vector.tensor_copy(
                                    qsw[r0 : r0 + 32, :], q_raw[r1 : r1 + 32, :]
                                )
                            nc.vector.tensor_tensor(
                                t2[:], qsw[:], sin_sb[:, nsl], mult_op
                            )
                            nc.vector.tensor_tensor(
                                QT_sb[:, m, nsl], t1[:], t2[:], add_op
                            )
                        else:
                            # rows 0:64 = K^T (rope), rows 64:128 = V^T (copy)
                            q_raw = wpool.tile([128, QTS], bf16, tag="qraw")
                            nc.scalar.activation(q_raw[0:64, :], ps[0:64, :], Copy)
                            t1 = wpool.tile([128, QTS], bf16, tag="t1")
                            t2 = wpool.tile([128, QTS], bf16, tag="t2")
                            nc.vector.tensor_tensor(
                                t1[0:64, :], q_raw[0:64, :], cos_sb[0:64, nsl], mult_op
                            )
                            qsw = wpool.tile([128, QTS], bf16, tag="qsw")
                            for r0, r1 in ((0, 32), (32, 0)):
                                nc.vector.tensor_copy(
                                    qsw[r0 : r0 + 32, :], q_raw[r1 : r1 + 32, :]
                                )
                            nc.vector.tensor_tensor(
                                t2[0:64, :], qsw[0:64, :], sin_sb[0:64, nsl], mult_op
                            )
                            nc.vector.tensor_tensor(
                                KT2_sb[0:64, nsl], t1[0:64, :], t2[0:64, :], add_op
                            )
                            # duplicate K^T into partitions 64:128 (row-group packing)
                            nc.vector.tensor_copy(
                                KT2_sb[64:128, nsl], KT2_sb[0:64, nsl]
                            )
                            # V^T: plain cast copy into partitions 64:128
                            nc.scalar.activation(
                                VT_sb[64:128, nsl], ps[64:128, :], Copy
                            )
                    if m == 2:
                        # V^T -> V (token-major) via DMA transpose
                        mark(f"b{b}_vtr")
                        for kt in range(n_skt):
                            nc.sync.dma_start_transpose(
                                V_sb[:, kt, 0:64],
                                VT_sb[64:128, kt * KTS : (kt + 1) * KTS],
                            )
                        mark(f"b{b}_proj2")

            def attn(b):
                mark(f"b{b}_attn")
                QT_sb = tiles[("QT", b)]
                KT2_sb = tiles[("KT2", b)]
                V_sb = tiles[("V", b)]
                OT_sb = bpool.tile([128, n_mo, s], bf16, tag="OT")
                tiles[("OT", b)] = OT_sb
                pending = []

                def normalize(hb2, m2, qsl2, ops2, rt2):
                    # recip already issued (DVE); broadcast + scale into OT
                    bps = psw.tile([128, QTS], f32, tag="w")
                    nc.tensor.matmul(
                        bps[0:64, :], ones_sb[:], rt2[:], start=True, stop=True
                    )
                    bsb = wpool.tile([64, QTS], f32, tag="bsb")
                    nc.any.tensor_copy(bsb[:], bps[0:64, :])
                    nc.vector.tensor_tensor(
                        OT_sb[hb2 : hb2 + 64, m2, qsl2],
                        ops2[0:64, :],
                        bsb[:],
                        mult_op,
                    )

                for qt in range(n_qt):
                    for h in range(HQ):
                        hb = (h % 2) * 64
                        qh = QT_sb[hb : hb + 64, h // 2, :]
                        kt2 = KT2_sb[hb : hb + 64, :]
                        qsl = slice(qt * QTS, (qt + 1) * QTS)
                        n_kt = (qt + 1) * (QTS // KTS)  # k tiles needed
                        ops = psops.tile([128, QTS], f32, tag="ops")
                        for g in range(0, n_kt, GRP):
                            kts = list(range(g, min(g + GRP, n_kt)))
                            sc = pssc.tile([128, GRP * QTS], f32, tag="sc")
                            e = epool.tile([128, GRP * QTS], bf16, tag="e")
                            for j, kt in enumerate(kts):
                                nc.tensor.matmul(
                                    sc[:, j * QTS : (j + 1) * QTS],
                                    kt2[:, kt * KTS : (kt + 1) * KTS],
                                    qh[:, qsl],
                                    start=True,
                                    stop=True,
                                )
                                o = kt * KTS - qt * QTS
                                if o >= 0:  # diagonal tile
                                    nc.vector.tensor_tensor(
                                        sc[:, j * QTS + o : j * QTS + o + 128],
                                        sc[:, j * QTS + o : j * QTS + o + 128],
                                        tri_sb[:],
                                        add_op,
                                    )
                            if g == 0 and pending:
                                # normalize the previous q-tile now; its recip
                                # had time to finish, so PE doesn't stall
                                normalize(*pending.pop())
                            nc.scalar.activation(
                                e[:, 0 : len(kts) * QTS], sc[:, 0 : len(kts) * QTS], Exp
                            )
                            for j, kt in enumerate(kts):
                                o = kt * KTS - qt * QTS
                                if o > 0:
                                    nc.gpsimd.memset(
                                        e[:, j * QTS : j * QTS + o], 0.0
                                    )
                                nc.tensor.matmul(
                                    ops[:],
                                    V_sb[:, kt, :],
                                    e[:, j * QTS : (j + 1) * QTS],
                                    start=(kt == 0),
                                    stop=(kt == n_kt - 1),
                                )
                        rt = wpool.tile([1, QTS], f32, tag="rt")
                        nc.vector.reciprocal(rt[:], ops[64:65, :])
                        pending.append((hb, h // 2, qsl, ops, rt))
                    if qt > 0:
                        wo_block(b, qt - 1)
                if pending:
                    normalize(*pending.pop())
                wo_block(b, n_qt - 1)

            def wo_block(b, qt):
                OT_sb = tiles[("OT", b)]
                for mt in range(4 * qt, 4 * qt + 4):
                    msl = slice(mt * 128, (mt + 1) * 128)
                    osb = opool.tile([128, d], bf16, tag="osb")
                    for n in range(d // QTS):
                        nsl = slice(n * QTS, (n + 1) * QTS)
                        pool = psw if n % 2 == 0 else pssc
                        ps = pool.tile([128, QTS], f32, tag="w" if n % 2 == 0 else "sc")
                        for kt in range(n_mo):
                            nc.tensor.matmul(
                                ps[:],
                                OT_sb[:, kt, msl],
                                wo_sb[:, kt, nsl],
                                start=(kt == 0),
                                stop=(kt == n_mo - 1),
                            )
                        nc.any.tensor_copy(osb[:, nsl], ps[:])
                    nc.sync.dma_start(
                        part_d[b * s + mt * 128 : b * s + (mt + 1) * 128, :],
                        osb[:],
                    )

            load_x(0)
            proj(0)
            load_x(1)  # b1 input load overlaps b0 attention (SP order)
            attn(0)
            proj(1)
            attn(1)
    mark("end")
    nc.compile()
    return nc


# ---------------- host-side sharding ----------------

_PERM = np.concatenate([np.arange(0, HD, 2), np.arange(1, HD, 2)])  # evens, odds


def make_core_inputs(x, freqs_cos, freqs_sin, wq, wk, wv, wo, s=S, d=D):
    """Build per-core input maps (list of dicts, one per core)."""
    b = x.shape[0]
    xT = np.ascontiguousarray(np.transpose(x, (0, 2, 1))).astype(BF16)  # [B, D, S]

    cosT = np.ascontiguousarray(freqs_cos.T)  # [32, S]
    sinT = np.ascontiguousarray(freqs_sin.T)
    cosb = np.tile(np.concatenate([cosT, cosT], axis=0), (2, 1)).astype(BF16)  # [128,S]
    sinb = np.tile(
        np.concatenate([-sinT, sinT], axis=0), (2, 1)
    ).astype(BF16)

    p = np.arange(128)[:, None]
    f = np.arange(128)[None, :]
    trimask = np.where(f >= p, 0.0, -1e9).astype(np.float32)

    scale = 1.0 / math.sqrt(HD)
    in_maps = []
    for c in range(N_CORES):
        wq_c = np.concatenate(
            [
                wq[:, (4 * c + h) * HD : (4 * c + h + 1) * HD][:, _PERM]
                for h in range(HQ)
            ],
            axis=1,
        ) * scale
        wk_c = wk[:, c * HD : (c + 1) * HD][:, _PERM]
        wv_c = wv[:, c * HD : (c + 1) * HD]
        wqkv = np.concatenate([wq_c, wk_c, wv_c], axis=1).astype(BF16)  # [D, 384]
        wo_c = np.ascontiguousarray(
            wo[4 * c * HD : (4 * c + HQ) * HD, :]
        ).astype(BF16)  # [256, D] — O is in original d-order (V unpermuted)
        in_maps.append(
            {
                "xT": xT,
                "wqkv": wqkv,
                "wo_s": wo_c,
                "cosb": cosb,
                "sinb": sinb,
                "trimask": trimask,
            }
        )
    return in_maps


_NC_CACHE = {}


def kernel(x, freqs_cos, freqs_sin, wq, wk, wv, wo):
    from concourse.bass_utils import run_bass_kernel_spmd

    x = np.asarray(x, np.float32)
    freqs_cos = np.asarray(freqs_cos, np.float32)
    freqs_sin = np.asarray(freqs_sin, np.float32)
    wq = np.asarray(wq, np.float32)
    wk = np.asarray(wk, np.float32)
    wv = np.asarray(wv, np.float32)
    wo = np.asarray(wo, np.float32)

    if "nc" not in _NC_CACHE:
        _NC_CACHE["nc"] = build_program()
    nc = _NC_CACHE["nc"]

    in_maps = make_core_inputs(x, freqs_cos, freqs_sin, wq, wk, wv, wo)
    res = run_bass_kernel_spmd(nc, in_maps, list(range(N_CORES)))
    acc = np.zeros((B * S, D), np.float32)
    for r in res.results:
        acc += np.asarray(r["part"], np.float32)
    return acc.reshape(B, S, D).astype(BF16)


# revision 18
# speedup vs baseline: 1.3419x; 1.0121x over previous
"""Trainium2 Bass kernel for nn_Attention_78151224918608.

Dense transformer attention block: QKV proj + RoPE + GQA causal attention
+ output proj. Sharding: tensor-parallel over heads across 8 cores
(core c: Q heads 4c..4c+3, KV head c). Each core computes a partial
output (its heads through wo rows); host sums the 8 bf16 partials in
fp32 and casts to bf16.

Layout strategy (per core, per batch):
  - All matmul operands bf16; accumulation fp32 in PSUM.
  - Projections computed transposed: QKV^T[384, S] = wqkv^T @ x^T so that
    Q^T/K^T (head-dim on partitions) feed the scores matmul directly.
  - RoPE: even/odd pair interleave is folded into wq/wk/wo columns on the
    host (perm = evens-then-odds), turning the pair swap into a 32-row
    block swap done with a small permutation matmul on PE.
  - Scores computed transposed per (b,h): S^T[k,q] = K^T.T @ Q^T, so the
    softmax denominator and P@V both contract over k = partitions:
    PV lhsT = [V | ones-col] gives O^T rows 0:64 and sumexp in row 64.
  - Causal: only k-tiles <= q-tile are computed; diagonal 128x128 blocks
    get an additive triangular mask in PSUM before exp; fully-invalid
    column strips of the exp tile are memset to 0 afterwards.
  - exp on ScalarE reads PSUM strips [128, 1024] and writes bf16 SBUF.
  - Normalization: recip = 1/sumexp (DVE), broadcast across 64 partitions
    with a K=1 ones matmul, multiplied into O^T during evacuation.
"""

import sys

sys.path.insert(0, "/opt/trn_rl_repo")

import math
import numpy as np
import ml_dtypes

BF16 = ml_dtypes.bfloat16

# Problem constants (hardcoded per contract).
B = 2
S = 2048
D = 2048
N_HEADS = 32
N_KV_HEADS = 8
HD = 64
N_CORES = 8
HQ = N_HEADS // N_CORES  # 4 q heads per core
M_PROJ = HQ * HD + 2 * HD  # 384: [Q0 Q1 Q2 Q3 | K | V]
QTS = 512  # q tile size (free dim)
KTS = 128  # k tile size (partitions)
GRP = 2  # k-tiles per exp strip


def build_program(s=S, d=D, phase_log=None):
    import concourse.bass as bass
    import concourse.mybir as mybir
    import concourse.tile as tile
    from concourse import bacc

    def mark(label):
        if phase_log is not None:
            phase_log.append((label, len(nc.inst_map)))

    f32 = mybir.dt.float32
    bf16 = mybir.dt.bfloat16
    Exp = mybir.ActivationFunctionType.Exp
    Copy = mybir.ActivationFunctionType.Copy
    add_op = mybir.AluOpType.add
    mult_op = mybir.AluOpType.mult

    n_qt = s // QTS  # q tiles per batch
    n_dkt = d // 128  # contraction tiles for projections
    n_skt = s // KTS  # k tiles per batch
    n_nt = s // QTS  # token tiles (512) for proj free dim
    n_mo = (HQ * HD) // 128  # wo contraction tiles (2)

    nc = bacc.Bacc("TRN2", num_devices=N_CORES)
    xT_d = nc.declare_dram_parameter("xT", [B, d, s], bf16, isOutput=False)
    wqkv_d = nc.declare_dram_parameter("wqkv", [d, M_PROJ], bf16, isOutput=False)
    wo_d = nc.declare_dram_parameter("wo_s", [HQ * HD, d], bf16, isOutput=False)
    cos_d = nc.declare_dram_parameter("cosb", [128, s], bf16, isOutput=False)
    sin_d = nc.declare_dram_parameter("sinb", [128, s], bf16, isOutput=False)
    tri_d = nc.declare_dram_parameter("trimask", [128, 128], f32, isOutput=False)
    part_d = nc.declare_dram_parameter("part", [B * s, d], bf16, isOutput=True)

    with tile.TileContext(nc) as tc:
        with (
            tc.tile_pool(name="const", bufs=1) as cpool,
            tc.tile_pool(name="big", bufs=1) as bpool,
            tc.tile_pool(name="work", bufs=3) as wpool,
            tc.tile_pool(name="estrip", bufs=3) as epool,
            tc.tile_pool(name="outp", bufs=3) as opool,
            tc.tile_pool(name="psw", bufs=3, space="PSUM") as psw,
            tc.tile_pool(name="pssc", bufs=2, space="PSUM") as pssc,
            tc.tile_pool(name="psops", bufs=1, space="PSUM") as psops,
        ):
            # ---- constants / weights ----
            cos_sb = cpool.tile([128, s], bf16)
            sin_sb = cpool.tile([128, s], bf16)
            tri_sb = cpool.tile([128, 128], f32)
            ones_sb = cpool.tile([1, 64], f32)
            wqkv_sb = cpool.tile([128, n_dkt, M_PROJ], bf16)
            wo_sb = cpool.tile([128, n_mo, d], bf16)

            nc.sync.dma_start(cos_sb[:], cos_d[:])
            nc.sync.dma_start(sin_sb[:], sin_d[:])
            nc.sync.dma_start(tri_sb[:], tri_d[:])
            nc.gpsimd.memset(ones_sb[:], 1.0)
            for kt in range(n_dkt):
                nc.sync.dma_start(
                    wqkv_sb[:, kt, :], wqkv_d[kt * 128 : (kt + 1) * 128, :]
                )
            for kt in range(n_mo):
                nc.sync.dma_start(wo_sb[:, kt, :], wo_d[kt * 128 : (kt + 1) * 128, :])

            # ---- per-batch persistent tiles ----
            tiles = {}

            def load_x(b):
                xT_sb = bpool.tile([128, n_dkt, s], bf16, tag="xT")
                tiles[("x", b)] = xT_sb
                for kt in range(n_dkt):
                    nc.sync.dma_start(
                        xT_sb[:, kt, :],
                        xT_d[b, kt * 128 : (kt + 1) * 128, :],
                    )

            def proj(b):
                mark(f"b{b}_proj")
                xT_sb = tiles[("x", b)]
                QT_sb = bpool.tile([128, n_mo, s], bf16, tag="QT")
                KT2_sb = bpool.tile([128, s], bf16, tag="KT2")
                VT_sb = bpool.tile([128, s], bf16, tag="VT")
                V_sb = bpool.tile([128, n_skt, 128], bf16, tag="V")
                tiles[("QT", b)] = QT_sb
                tiles[("KT2", b)] = KT2_sb
                tiles[("V", b)] = V_sb
                # ones column / zero pad for PV lhsT
                nc.gpsimd.memset(V_sb[:, :, 64:128], 0.0)
                nc.gpsimd.memset(V_sb[:, :, 64:65], 1.0)
                for m in (2, 0, 1):  # K/V first so attention can start early
                    for n in range(n_nt):
                        nsl = slice(n * QTS, (n + 1) * QTS)
                        ps = psw.tile([128, QTS], f32, tag="w")
                        for kt in range(n_dkt):
                            nc.tensor.matmul(
                                ps[:],
                                wqkv_sb[:, kt, m * 128 : (m + 1) * 128],
                                xT_sb[:, kt, nsl],
                                start=(kt == 0),
                                stop=(kt == n_dkt - 1),
                            )
                        if m < 2:
                            # two Q heads stacked: rope all 128 rows
                            q_raw = wpool.tile([128, QTS], bf16, tag="qraw")
                            nc.scalar.activation(q_raw[:], ps[:], Copy)
                            t1 = wpool.tile([128, QTS], bf16, tag="t1")
                            t2 = wpool.tile([128, QTS], bf16, tag="t2")
                            nc.vector.tensor_tensor(
                                t1[:], q_raw[:], cos_sb[:, nsl], mult_op
                            )
                            # swap(q_raw) via cross-base copies, then * sin
                            qsw = wpool.tile([128, QTS], bf16, tag="qsw")
                            for r0, r1 in ((0, 32), (32, 0), (64, 96), (96, 64)):
                                nc.vector.tensor_copy(
                                    qsw[r0 : r0 + 32, :], q_raw[r1 : r1 + 32, :]
                                )
                            nc.vector.tensor_tensor(
                                t2[:], qsw[:], sin_sb[:, nsl], mult_op
                            )
                            nc.vector.tensor_tensor(
                                QT_sb[:, m, nsl], t1[:], t2[:], add_op
                            )
                        else:
                            # rows 0:64 = K^T (rope), rows 64:128 = V^T (copy)
                            q_raw = wpool.tile([128, QTS], bf16, tag="qraw")
                            nc.scalar.activation(q_raw[0:64, :], ps[0:64, :], Copy)
                            t1 = wpool.tile([128, QTS], bf16, tag="t1")
                            t2 = wpool.tile([128, QTS], bf16, tag="t2")
                            nc.vector.tensor_tensor(
                                t1[0:64, :], q_raw[0:64, :], cos_sb[0:64, nsl], mult_op
                            )
                            qsw = wpool.tile([128, QTS], bf16, tag="qsw")
                            for r0, r1 in ((0, 32), (32, 0)):
                                nc.vector.tensor_copy(
                                    qsw[r0 : r0 + 32, :], q_raw[r1 : r1 + 32, :]
                                )
                            nc.vector.tensor_tensor(
                                t2[0:64, :], qsw[0:64, :], sin_sb[0:64, nsl], mult_op
                            )
                            nc.vector.tensor_tensor(
                                KT2_sb[0:64, nsl], t1[0:64, :], t2[0:64, :], add_op
                            )
                            # duplicate K^T into partitions 64:128 (row-group packing)
                            nc.vector.tensor_copy(
                                KT2_sb[64:128, nsl], KT2_sb[0:64, nsl]
                            )
                            # V^T: plain cast copy into partitions 64:128
                            nc.scalar.activation(
                                VT_sb[64:128, nsl], ps[64:128, :], Copy
                            )
                    if m == 2:
                        # V^T -> V (token-major) via DMA transpose
                        mark(f"b{b}_vtr")
                        for kt in range(n_skt):
                            nc.sync.dma_start_transpose(
                                V_sb[:, kt, 0:64],
                                VT_sb[64:128, kt * KTS : (kt + 1) * KTS],
                            )
                        mark(f"b{b}_proj2")

            def attn(b):
                mark(f"b{b}_attn")
                QT_sb = tiles[("QT", b)]
                KT2_sb = tiles[("KT2", b)]
                V_sb = tiles[("V", b)]
                OT_sb = bpool.tile([128, n_mo, s], bf16, tag="OT")
                tiles[("OT", b)] = OT_sb
                pending = []

                def normalize(hb2, m2, qsl2, ops2, rt2):
                    # recip already issued (DVE); broadcast + scale into OT
                    bps = psw.tile([128, QTS], f32, tag="w")
                    nc.tensor.matmul(
                        bps[0:64, :], ones_sb[:], rt2[:], start=True, stop=True
                    )
                    bsb = wpool.tile([64, QTS], f32, tag="bsb")
                    nc.any.tensor_copy(bsb[:], bps[0:64, :])
                    nc.vector.tensor_tensor(
                        OT_sb[hb2 : hb2 + 64, m2, qsl2],
                        ops2[0:64, :],
                        bsb[:],
                        mult_op,
                    )

                for qt in range(n_qt):
                    for h in range(HQ):
                        hb = (h % 2) * 64
                        qh = QT_sb[hb : hb + 64, h // 2, :]
                        kt2 = KT2_sb[hb : hb + 64, :]
                        qsl = slice(qt * QTS, (qt + 1) * QTS)
                        n_kt = (qt + 1) * (QTS // KTS)  # k tiles needed
                        ops = psops.tile([128, QTS], f32, tag="ops")
                        for g in range(0, n_kt, GRP):
                            kts = list(range(g, min(g + GRP, n_kt)))
                            sc = pssc.tile([128, GRP * QTS], f32, tag="sc")
                            e = epool.tile([128, GRP * QTS], bf16, tag="e")
                            for j, kt in enumerate(kts):
                                nc.tensor.matmul(
                                    sc[:, j * QTS : (j + 1) * QTS],
                                    kt2[:, kt * KTS : (kt + 1) * KTS],
                                    qh[:, qsl],
                                    start=True,
                                    stop=True,
                                )
                                o = kt * KTS - qt * QTS
                                if o >= 0:  # diagonal tile
                                    nc.vector.tensor_tensor(
                                        sc[:, j * QTS + o : j * QTS + o + 128],
                                        sc[:, j * QTS + o : j * QTS + o + 128],
                                        tri_sb[:],
                                        add_op,
                                    )
                            if g == 0 and pending:
                                # normalize the previous q-tile now; its recip
                                # had time to finish, so PE doesn't stall
                                normalize(*pending.pop())
                            nc.scalar.activation(
                                e[:, 0 : len(kts) * QTS], sc[:, 0 : len(kts) * QTS], Exp
                            )
                            for j, kt in enumerate(kts):
                                o = kt * KTS - qt * QTS
                                if o > 0:
                                    nc.gpsimd.memset(
                                        e[:, j * QTS : j * QTS + o], 0.0
                                    )
                                nc.tensor.matmul(
                                    ops[:],
                                    V_sb[:, kt, :],
                                    e[:, j * QTS : (j + 1) * QTS],
                                    start=(kt == 0),
                                    stop=(kt == n_kt - 1),
                                )
                        rt = wpool.tile([1, QTS], f32, tag="rt")
                        nc.vector.reciprocal(rt[:], ops[64:65, :])
                        pending.append((hb, h // 2, qsl, ops, rt))
                    if qt > 0:
                        wo_block(b, qt - 1)
                if pending:
                    normalize(*pending.pop())
                wo_block(b, n_qt - 1)

            def wo_block(b, qt):
                OT_sb = tiles[("OT", b)]
                for mt in range(4 * qt, 4 * qt + 4):
                    msl = slice(mt * 128, (mt + 1) * 128)
                    osb = opool.tile([128, d], bf16, tag="osb")
                    for n in range(d // QTS):
                        nsl = slice(n * QTS, (n + 1) * QTS)
                        pool = psw if n % 2 == 0 else pssc
                        ps = pool.tile([128, QTS], f32, tag="w" if n % 2 == 0 else "sc")
                        for kt in range(n_mo):
                            nc.tensor.matmul(
                                ps[:],
                                OT_sb[:, kt, msl],
                                wo_sb[:, kt, nsl],
                                start=(kt == 0),
                                stop=(kt == n_mo - 1),
                            )
                        nc.any.tensor_copy(osb[:, nsl], ps[:])
                    nc.sync.dma_start(
                        part_d[b * s + mt * 128 : b * s + (mt + 1) * 128, :],
                        osb[:],
                    )

            load_x(0)
            proj(0)
            load_x(1)  # b1 input load overlaps b0 attention (SP order)
            attn(0)
            proj(1)
            attn(1)
    mark("end")
    nc.compile()
    return nc


# ---------------- host-side sharding ----------------

_PERM = np.concatenate([np.arange(0, HD, 2), np.arange(1, HD, 2)])  # evens, odds


def make_core_inputs(x, freqs_cos, freqs_sin, wq, wk, wv, wo, s=S, d=D):
    """Build per-core input maps (list of dicts, one per core)."""
    b = x.shape[0]
    xT = np.ascontiguousarray(np.transpose(x, (0, 2, 1))).astype(BF16)  # [B, D, S]

    cosT = np.ascontiguousarray(freqs_cos.T)  # [32, S]
    sinT = np.ascontiguousarray(freqs_sin.T)
    cosb = np.tile(np.concatenate([cosT, cosT], axis=0), (2, 1)).astype(BF16)  # [128,S]
    sinb = np.tile(
        np.concatenate([-sinT, sinT], axis=0), (2, 1)
    ).astype(BF16)

    p = np.arange(128)[:, None]
    f = np.arange(128)[None, :]
    trimask = np.where(f >= p, 0.0, -1e9).astype(np.float32)

    scale = 1.0 / math.sqrt(HD)
    in_maps = []
    for c in range(N_CORES):
        wq_c = np.concatenate(
            [
                wq[:, (4 * c + h) * HD : (4 * c + h + 1) * HD][:, _PERM]
                for h in range(HQ)
            ],
            axis=1,
        ) * scale
        wk_c = wk[:, c * HD : (c + 1) * HD][:, _PERM]
        wv_c = wv[:, c * HD : (c + 1) * HD]
        wqkv = np.concatenate([wq_c, wk_c, wv_c], axis=1).astype(BF16)  # [D, 384]
        wo_c = np.ascontiguousarray(
            wo[4 * c * HD : (4 * c + HQ) * HD, :]
        ).astype(BF16)  # [256, D] — O is in original d-order (V unpermuted)
        in_maps.append(
            {
                "xT": xT,
                "wqkv": wqkv,
                "wo_s": wo_c,
                "cosb": cosb,
                "sinb": sinb,
                "trimask": trimask,
            }
        )
    return in_maps


_NC_CACHE = {}


def kernel(x, freqs_cos, freqs_sin, wq, wk, wv, wo):
    from concourse.bass_utils import run_bass_kernel_spmd

    x = np.asarray(x, np.float32)
    freqs_cos = np.asarray(freqs_cos, np.float32)
    freqs_sin = np.asarray(freqs_sin, np.float32)
    wq = np.asarray(wq, np.float32)
    wk = np.asarray(wk, np.float32)
    wv = np.asarray(wv, np.float32)
    wo = np.asarray(wo, np.float32)

    if "nc" not in _NC_CACHE:
        _NC_CACHE["nc"] = build_program()
    nc = _NC_CACHE["nc"]

    in_maps = make_core_inputs(x, freqs_cos, freqs_sin, wq, wk, wv, wo)
    res = run_bass_kernel_spmd(nc, in_maps, list(range(N_CORES)))
    acc = np.zeros((B * S, D), np.float32)
    for r in res.results:
        acc += np.asarray(r["part"], np.float32)
    return acc.reshape(B, S, D).astype(BF16)


# revision 19
# speedup vs baseline: 1.3796x; 1.0281x over previous
"""Trainium2 Bass kernel for nn_Attention_78151224918608.

Dense transformer attention block: QKV proj + RoPE + GQA causal attention
+ output proj. Sharding: tensor-parallel over heads across 8 cores
(core c: Q heads 4c..4c+3, KV head c). Each core computes a partial
output (its heads through wo rows); host sums the 8 bf16 partials in
fp32 and casts to bf16.

Layout strategy (per core, per batch):
  - All matmul operands bf16; accumulation fp32 in PSUM.
  - Projections computed transposed: QKV^T[384, S] = wqkv^T @ x^T so that
    Q^T/K^T (head-dim on partitions) feed the scores matmul directly.
  - RoPE: even/odd pair interleave is folded into wq/wk/wo columns on the
    host (perm = evens-then-odds), turning the pair swap into a 32-row
    block swap done with a small permutation matmul on PE.
  - Scores computed transposed per (b,h): S^T[k,q] = K^T.T @ Q^T, so the
    softmax denominator and P@V both contract over k = partitions:
    PV lhsT = [V | ones-col] gives O^T rows 0:64 and sumexp in row 64.
  - Causal: only k-tiles <= q-tile are computed; diagonal 128x128 blocks
    get an additive triangular mask in PSUM before exp; fully-invalid
    column strips of the exp tile are memset to 0 afterwards.
  - exp on ScalarE reads PSUM strips [128, 1024] and writes bf16 SBUF.
  - Normalization: recip = 1/sumexp (DVE), broadcast across 64 partitions
    with a K=1 ones matmul, multiplied into O^T during evacuation.
"""

import sys

sys.path.insert(0, "/opt/trn_rl_repo")

import math
import numpy as np
import ml_dtypes

BF16 = ml_dtypes.bfloat16

# Problem constants (hardcoded per contract).
B = 2
S = 2048
D = 2048
N_HEADS = 32
N_KV_HEADS = 8
HD = 64
N_CORES = 8
HQ = N_HEADS // N_CORES  # 4 q heads per core
M_PROJ = HQ * HD + 2 * HD  # 384: [Q0 Q1 Q2 Q3 | K | V]
QTS = 512  # q tile size (free dim)
KTS = 128  # k tile size (partitions)
GRP = 2  # k-tiles per exp strip


def build_program(s=S, d=D, phase_log=None):
    import concourse.bass as bass
    import concourse.mybir as mybir
    import concourse.tile as tile
    from concourse import bacc

    def mark(label):
        if phase_log is not None:
            phase_log.append((label, len(nc.inst_map)))

    f32 = mybir.dt.float32
    bf16 = mybir.dt.bfloat16
    Exp = mybir.ActivationFunctionType.Exp
    Copy = mybir.ActivationFunctionType.Copy
    add_op = mybir.AluOpType.add
    mult_op = mybir.AluOpType.mult

    n_qt = s // QTS  # q tiles per batch
    n_dkt = d // 128  # contraction tiles for projections
    n_skt = s // KTS  # k tiles per batch
    n_nt = s // QTS  # token tiles (512) for proj free dim
    n_mo = (HQ * HD) // 128  # wo contraction tiles (2)

    nc = bacc.Bacc("TRN2", num_devices=N_CORES)
    xT_d = nc.declare_dram_parameter("xT", [B, d, s], bf16, isOutput=False)
    wqkv_d = nc.declare_dram_parameter("wqkv", [d, M_PROJ], bf16, isOutput=False)
    wo_d = nc.declare_dram_parameter("wo_s", [HQ * HD, d], bf16, isOutput=False)
    cos_d = nc.declare_dram_parameter("cosb", [128, s], bf16, isOutput=False)
    sin_d = nc.declare_dram_parameter("sinb", [128, s], bf16, isOutput=False)
    tri_d = nc.declare_dram_parameter("trimask", [128, 128], f32, isOutput=False)
    part_d = nc.declare_dram_parameter("part", [B * s, d], bf16, isOutput=True)

    with tile.TileContext(nc) as tc:
        with (
            tc.tile_pool(name="const", bufs=1) as cpool,
            tc.tile_pool(name="big", bufs=1) as bpool,
            tc.tile_pool(name="work", bufs=3) as wpool,
            tc.tile_pool(name="estrip", bufs=5) as epool,
            tc.tile_pool(name="outp", bufs=4) as opool,
            tc.tile_pool(name="psw", bufs=3, space="PSUM") as psw,
            tc.tile_pool(name="pssc", bufs=2, space="PSUM") as pssc,
            tc.tile_pool(name="psops", bufs=1, space="PSUM") as psops,
        ):
            # ---- constants / weights ----
            cos_sb = cpool.tile([128, s], bf16)
            sin_sb = cpool.tile([128, s], bf16)
            tri_sb = cpool.tile([128, 128], f32)
            ones_sb = cpool.tile([1, 64], f32)
            wqkv_sb = cpool.tile([128, n_dkt, M_PROJ], bf16)
            wo_sb = cpool.tile([128, n_mo, d], bf16)

            nc.sync.dma_start(cos_sb[:], cos_d[:])
            nc.sync.dma_start(sin_sb[:], sin_d[:])
            nc.sync.dma_start(tri_sb[:], tri_d[:])
            nc.gpsimd.memset(ones_sb[:], 1.0)
            for kt in range(n_dkt):
                nc.sync.dma_start(
                    wqkv_sb[:, kt, :], wqkv_d[kt * 128 : (kt + 1) * 128, :]
                )
            for kt in range(n_mo):
                nc.sync.dma_start(wo_sb[:, kt, :], wo_d[kt * 128 : (kt + 1) * 128, :])

            # ---- per-batch persistent tiles ----
            tiles = {}

            def load_x(b):
                xT_sb = bpool.tile([128, n_dkt, s], bf16, tag="xT")
                tiles[("x", b)] = xT_sb
                for kt in range(n_dkt):
                    nc.sync.dma_start(
                        xT_sb[:, kt, :],
                        xT_d[b, kt * 128 : (kt + 1) * 128, :],
                    )

            def proj(b):
                mark(f"b{b}_proj")
                xT_sb = tiles[("x", b)]
                QT_sb = bpool.tile([128, n_mo, s], bf16, tag="QT")
                KT2_sb = bpool.tile([128, s], bf16, tag="KT2")
                VT_sb = bpool.tile([128, s], bf16, tag="VT")
                V_sb = bpool.tile([128, n_skt, 128], bf16, tag="V")
                tiles[("QT", b)] = QT_sb
                tiles[("KT2", b)] = KT2_sb
                tiles[("V", b)] = V_sb
                # ones column / zero pad for PV lhsT
                nc.gpsimd.memset(V_sb[:, :, 64:128], 0.0)
                nc.gpsimd.memset(V_sb[:, :, 64:65], 1.0)
                for m in (2, 0, 1):  # K/V first so attention can start early
                    for n in range(n_nt):
                        nsl = slice(n * QTS, (n + 1) * QTS)
                        ps = psw.tile([128, QTS], f32, tag="w")
                        for kt in range(n_dkt):
                            nc.tensor.matmul(
                                ps[:],
                                wqkv_sb[:, kt, m * 128 : (m + 1) * 128],
                                xT_sb[:, kt, nsl],
                                start=(kt == 0),
                                stop=(kt == n_dkt - 1),
                            )
                        if m < 2:
                            # two Q heads stacked: rope all 128 rows
                            q_raw = wpool.tile([128, QTS], bf16, tag="qraw")
                            nc.scalar.activation(q_raw[:], ps[:], Copy)
                            t1 = wpool.tile([128, QTS], bf16, tag="t1")
                            t2 = wpool.tile([128, QTS], bf16, tag="t2")
                            nc.vector.tensor_tensor(
                                t1[:], q_raw[:], cos_sb[:, nsl], mult_op
                            )
                            # swap(q_raw) via cross-base copies, then * sin
                            qsw = wpool.tile([128, QTS], bf16, tag="qsw")
                            for r0, r1 in ((0, 32), (32, 0), (64, 96), (96, 64)):
                                nc.vector.tensor_copy(
                                    qsw[r0 : r0 + 32, :], q_raw[r1 : r1 + 32, :]
                                )
                            nc.vector.tensor_tensor(
                                t2[:], qsw[:], sin_sb[:, nsl], mult_op
                            )
                            nc.vector.tensor_tensor(
                                QT_sb[:, m, nsl], t1[:], t2[:], add_op
                            )
                        else:
                            # rows 0:64 = K^T (rope), rows 64:128 = V^T (copy)
                            q_raw = wpool.tile([128, QTS], bf16, tag="qraw")
                            nc.scalar.activation(q_raw[0:64, :], ps[0:64, :], Copy)
                            t1 = wpool.tile([128, QTS], bf16, tag="t1")
                            t2 = wpool.tile([128, QTS], bf16, tag="t2")
                            nc.vector.tensor_tensor(
                                t1[0:64, :], q_raw[0:64, :], cos_sb[0:64, nsl], mult_op
                            )
                            qsw = wpool.tile([128, QTS], bf16, tag="qsw")
                            for r0, r1 in ((0, 32), (32, 0)):
                                nc.vector.tensor_copy(
                                    qsw[r0 : r0 + 32, :], q_raw[r1 : r1 + 32, :]
                                )
                            nc.vector.tensor_tensor(
                                t2[0:64, :], qsw[0:64, :], sin_sb[0:64, nsl], mult_op
                            )
                            nc.vector.tensor_tensor(
                                KT2_sb[0:64, nsl], t1[0:64, :], t2[0:64, :], add_op
                            )
                            # duplicate K^T into partitions 64:128 (row-group packing)
                            nc.vector.tensor_copy(
                                KT2_sb[64:128, nsl], KT2_sb[0:64, nsl]
                            )
                            # V^T: plain cast copy into partitions 64:128
                            nc.scalar.activation(
                                VT_sb[64:128, nsl], ps[64:128, :], Copy
                            )
                    if m == 2:
                        # V^T -> V (token-major) via DMA transpose
                        mark(f"b{b}_vtr")
                        for kt in range(n_skt):
                            nc.sync.dma_start_transpose(
                                V_sb[:, kt, 0:64],
                                VT_sb[64:128, kt * KTS : (kt + 1) * KTS],
                            )
                        mark(f"b{b}_proj2")

            def attn(b):
                mark(f"b{b}_attn")
                QT_sb = tiles[("QT", b)]
                KT2_sb = tiles[("KT2", b)]
                V_sb = tiles[("V", b)]
                OT_sb = bpool.tile([128, n_mo, s], bf16, tag="OT")
                tiles[("OT", b)] = OT_sb
                pending = []

                def normalize(hb2, m2, qsl2, ops2, rt2):
                    # recip already issued (DVE); broadcast + scale into OT
                    bps = psw.tile([128, QTS], f32, tag="w")
                    nc.tensor.matmul(
                        bps[0:64, :], ones_sb[:], rt2[:], start=True, stop=True
                    )
                    bsb = wpool.tile([64, QTS], f32, tag="bsb")
                    nc.any.tensor_copy(bsb[:], bps[0:64, :])
                    nc.vector.tensor_tensor(
                        OT_sb[hb2 : hb2 + 64, m2, qsl2],
                        ops2[0:64, :],
                        bsb[:],
                        mult_op,
                    )

                for qt in range(n_qt):
                    for h in range(HQ):
                        hb = (h % 2) * 64
                        qh = QT_sb[hb : hb + 64, h // 2, :]
                        kt2 = KT2_sb[hb : hb + 64, :]
                        qsl = slice(qt * QTS, (qt + 1) * QTS)
                        n_kt = (qt + 1) * (QTS // KTS)  # k tiles needed
                        ops = psops.tile([128, QTS], f32, tag="ops")
                        for g in range(0, n_kt, GRP):
                            kts = list(range(g, min(g + GRP, n_kt)))
                            sc = pssc.tile([128, GRP * QTS], f32, tag="sc")
                            e = epool.tile([128, GRP * QTS], bf16, tag="e")
                            for j, kt in enumerate(kts):
                                nc.tensor.matmul(
                                    sc[:, j * QTS : (j + 1) * QTS],
                                    kt2[:, kt * KTS : (kt + 1) * KTS],
                                    qh[:, qsl],
                                    start=True,
                                    stop=True,
                                )
                                o = kt * KTS - qt * QTS
                                if o >= 0:  # diagonal tile
                                    nc.vector.tensor_tensor(
                                        sc[:, j * QTS + o : j * QTS + o + 128],
                                        sc[:, j * QTS + o : j * QTS + o + 128],
                                        tri_sb[:],
                                        add_op,
                                    )
                            if g == 0 and pending:
                                # normalize the previous q-tile now; its recip
                                # had time to finish, so PE doesn't stall
                                normalize(*pending.pop())
                            nc.scalar.activation(
                                e[:, 0 : len(kts) * QTS], sc[:, 0 : len(kts) * QTS], Exp
                            )
                            for j, kt in enumerate(kts):
                                o = kt * KTS - qt * QTS
                                if o > 0:
                                    nc.gpsimd.memset(
                                        e[:, j * QTS : j * QTS + o], 0.0
                                    )
                                nc.tensor.matmul(
                                    ops[:],
                                    V_sb[:, kt, :],
                                    e[:, j * QTS : (j + 1) * QTS],
                                    start=(kt == 0),
                                    stop=(kt == n_kt - 1),
                                )
                        rt = wpool.tile([1, QTS], f32, tag="rt")
                        nc.vector.reciprocal(rt[:], ops[64:65, :])
                        pending.append((hb, h // 2, qsl, ops, rt))
                    if qt > 0:
                        wo_block(b, qt - 1)
                if pending:
                    normalize(*pending.pop())
                wo_block(b, n_qt - 1)

            def wo_block(b, qt):
                OT_sb = tiles[("OT", b)]
                for mt in range(4 * qt, 4 * qt + 4):
                    msl = slice(mt * 128, (mt + 1) * 128)
                    osb = opool.tile([128, d], bf16, tag="osb")
                    for n in range(d // QTS):
                        nsl = slice(n * QTS, (n + 1) * QTS)
                        pool = psw if n % 2 == 0 else pssc
                        ps = pool.tile([128, QTS], f32, tag="w" if n % 2 == 0 else "sc")
                        for kt in range(n_mo):
                            nc.tensor.matmul(
                                ps[:],
                                OT_sb[:, kt, msl],
                                wo_sb[:, kt, nsl],
                                start=(kt == 0),
                                stop=(kt == n_mo - 1),
                            )
                        nc.any.tensor_copy(osb[:, nsl], ps[:])
                    nc.sync.dma_start(
                        part_d[b * s + mt * 128 : b * s + (mt + 1) * 128, :],
                        osb[:],
                    )

            load_x(0)
            proj(0)
            load_x(1)  # b1 input load overlaps b0 attention (SP order)
            attn(0)
            proj(1)
            attn(1)
    mark("end")
    nc.compile()
    return nc


# ---------------- host-side sharding ----------------

_PERM = np.concatenate([np.arange(0, HD, 2), np.arange(1, HD, 2)])  # evens, odds


def make_core_inputs(x, freqs_cos, freqs_sin, wq, wk, wv, wo, s=S, d=D):
    """Build per-core input maps (list of dicts, one per core)."""
    b = x.shape[0]
    xT = np.ascontiguousarray(np.transpose(x, (0, 2, 1))).astype(BF16)  # [B, D, S]

    cosT = np.ascontiguousarray(freqs_cos.T)  # [32, S]
    sinT = np.ascontiguousarray(freqs_sin.T)
    cosb = np.tile(np.concatenate([cosT, cosT], axis=0), (2, 1)).astype(BF16)  # [128,S]
    sinb = np.tile(
        np.concatenate([-sinT, sinT], axis=0), (2, 1)
    ).astype(BF16)

    p = np.arange(128)[:, None]
    f = np.arange(128)[None, :]
    trimask = np.where(f >= p, 0.0, -1e9).astype(np.float32)

    scale = 1.0 / math.sqrt(HD)
    in_maps = []
    for c in range(N_CORES):
        wq_c = np.concatenate(
            [
                wq[:, (4 * c + h) * HD : (4 * c + h + 1) * HD][:, _PERM]
                for h in range(HQ)
            ],
            axis=1,
        ) * scale
        wk_c = wk[:, c * HD : (c + 1) * HD][:, _PERM]
        wv_c = wv[:, c * HD : (c + 1) * HD]
        wqkv = np.concatenate([wq_c, wk_c, wv_c], axis=1).astype(BF16)  # [D, 384]
        wo_c = np.ascontiguousarray(
            wo[4 * c * HD : (4 * c + HQ) * HD, :]
        ).astype(BF16)  # [256, D] — O is in original d-order (V unpermuted)
        in_maps.append(
            {
                "xT": xT,
                "wqkv": wqkv,
                "wo_s": wo_c,
                "cosb": cosb,
                "sinb": sinb,
                "trimask": trimask,
            }
        )
    return in_maps


_NC_CACHE = {}


def kernel(x, freqs_cos, freqs_sin, wq, wk, wv, wo):
    from concourse.bass_utils import run_bass_kernel_spmd

    x = np.asarray(x, np.float32)
    freqs_cos = np.asarray(freqs_cos, np.float32)
    freqs_sin = np.asarray(freqs_sin, np.float32)
    wq = np.asarray(wq, np.float32)
    wk = np.asarray(wk, np.float32)
    wv = np.asarray(wv, np.float32)
    wo = np.asarray(wo, np.float32)

    if "nc" not in _NC_CACHE:
        _NC_CACHE["nc"] = build_program()
    nc = _NC_CACHE["nc"]

    in_maps = make_core_inputs(x, freqs_cos, freqs_sin, wq, wk, wv, wo)
    res = run_bass_kernel_spmd(nc, in_maps, list(range(N_CORES)))
    acc = np.zeros((B * S, D), np.float32)
    for r in res.results:
        acc += np.asarray(r["part"], np.float32)
    return acc.reshape(B, S, D).astype(BF16)


# revision 26
# speedup vs baseline: 1.4380x; 1.0423x over previous
"""Trainium2 Bass kernel for nn_Attention_78151224918608.

Dense transformer attention block: QKV proj + RoPE + GQA causal attention
+ output proj. Sharding: tensor-parallel over heads across 8 cores
(core c: Q heads 4c..4c+3, KV head c). Each core computes a partial
output (its heads through wo rows); host sums the 8 bf16 partials in
fp32 and casts to bf16.

Layout strategy (per core, per batch):
  - All matmul operands bf16; accumulation fp32 in PSUM.
  - Projections computed transposed: QKV^T[384, S] = wqkv^T @ x^T so that
    Q^T/K^T (head-dim on partitions) feed the scores matmul directly.
  - RoPE: even/odd pair interleave is folded into wq/wk/wo columns on the
    host (perm = evens-then-odds), turning the pair swap into a 32-row
    block swap done with a small permutation matmul on PE.
  - Scores computed transposed per (b,h): S^T[k,q] = K^T.T @ Q^T, so the
    softmax denominator and P@V both contract over k = partitions:
    PV lhsT = [V | ones-col] gives O^T rows 0:64 and sumexp in row 64.
  - Causal: only k-tiles <= q-tile are computed; diagonal 128x128 blocks
    get an additive triangular mask in PSUM before exp; fully-invalid
    column strips of the exp tile are memset to 0 afterwards.
  - exp on ScalarE reads PSUM strips [128, 1024] and writes bf16 SBUF.
  - Normalization: recip = 1/sumexp (DVE), broadcast across 64 partitions
    with a K=1 ones matmul, multiplied into O^T during evacuation.
"""

import sys

sys.path.insert(0, "/opt/trn_rl_repo")

import math
import numpy as np
import ml_dtypes

BF16 = ml_dtypes.bfloat16

# Problem constants (hardcoded per contract).
B = 2
S = 2048
D = 2048
N_HEADS = 32
N_KV_HEADS = 8
HD = 64
N_CORES = 8
HQ = N_HEADS // N_CORES  # 4 q heads per core
M_PROJ = HQ * HD + 2 * HD  # 384: [Q0 Q1 Q2 Q3 | K | V]
QTS = 512  # q tile size (free dim)
KTS = 128  # k tile size (partitions)
GRP = 2  # k-tiles per exp strip


def build_program(s=S, d=D, phase_log=None):
    import concourse.bass as bass
    import concourse.mybir as mybir
    import concourse.tile as tile
    from concourse import bacc

    def mark(label):
        if phase_log is not None:
            phase_log.append((label, len(nc.inst_map)))

    f32 = mybir.dt.float32
    bf16 = mybir.dt.bfloat16
    Exp = mybir.ActivationFunctionType.Exp
    Copy = mybir.ActivationFunctionType.Copy
    add_op = mybir.AluOpType.add
    mult_op = mybir.AluOpType.mult

    n_qt = s // QTS  # q tiles per batch
    n_dkt = d // 128  # contraction tiles for projections
    n_skt = s // KTS  # k tiles per batch
    n_nt = s // QTS  # token tiles (512) for proj free dim
    n_mo = (HQ * HD) // 128  # wo contraction tiles (2)

    nc = bacc.Bacc("TRN2", num_devices=N_CORES)
    xT_d = nc.declare_dram_parameter("xT", [B, d, s], bf16, isOutput=False)
    wqkv_d = nc.declare_dram_parameter("wqkv", [d, M_PROJ], bf16, isOutput=False)
    wo_d = nc.declare_dram_parameter("wo_s", [HQ * HD, d], bf16, isOutput=False)
    cos_d = nc.declare_dram_parameter("cosb", [128, s], bf16, isOutput=False)
    sin_d = nc.declare_dram_parameter("sinb", [128, s], bf16, isOutput=False)
    tri_d = nc.declare_dram_parameter("trimask", [128, 128], f32, isOutput=False)
    tri01_d = nc.declare_dram_parameter("tri01", [128, 4, QTS], bf16, isOutput=False)
    part_d = nc.declare_dram_parameter("part", [B * s, d], bf16, isOutput=True)

    with tile.TileContext(nc) as tc:
        with (
            tc.tile_pool(name="const", bufs=1) as cpool,
            tc.tile_pool(name="big", bufs=1) as bpool,
            tc.tile_pool(name="work", bufs=3) as wpool,
            tc.tile_pool(name="estrip", bufs=5) as epool,
            tc.tile_pool(name="outp", bufs=4) as opool,
            tc.tile_pool(name="psw", bufs=3, space="PSUM") as psw,
            tc.tile_pool(name="pssc", bufs=2, space="PSUM") as pssc,
            tc.tile_pool(name="psops", bufs=1, space="PSUM") as psops,
        ):
            # ---- constants / weights ----
            cos_sb = cpool.tile([128, s], bf16)
            sin_sb = cpool.tile([128, s], bf16)
            tri_sb = cpool.tile([128, 128], f32)
            tri01_sb = cpool.tile([128, 4, QTS], bf16)
            ones_sb = cpool.tile([1, 64], f32)
            wqkv_sb = cpool.tile([128, n_dkt, M_PROJ], bf16)
            wo_sb = cpool.tile([128, n_mo, d], bf16)

            nc.sync.dma_start(cos_sb[:], cos_d[:])
            nc.sync.dma_start(sin_sb[:], sin_d[:])
            nc.sync.dma_start(tri_sb[:], tri_d[:])
            nc.sync.dma_start(tri01_sb[:], tri01_d[:])
            nc.gpsimd.memset(ones_sb[:], 1.0)
            for kt in range(n_dkt):
                nc.sync.dma_start(
                    wqkv_sb[:, kt, :], wqkv_d[kt * 128 : (kt + 1) * 128, :]
                )
            for kt in range(n_mo):
                nc.sync.dma_start(wo_sb[:, kt, :], wo_d[kt * 128 : (kt + 1) * 128, :])

            # ---- per-batch persistent tiles ----
            tiles = {}

            def load_x(b):
                xT_sb = bpool.tile([128, n_dkt, s], bf16, tag="xT")
                tiles[("x", b)] = xT_sb
                for kt in range(n_dkt):
                    nc.sync.dma_start(
                        xT_sb[:, kt, :],
                        xT_d[b, kt * 128 : (kt + 1) * 128, :],
                    )

            def proj(b):
                mark(f"b{b}_proj")
                xT_sb = tiles[("x", b)]
                QT_sb = bpool.tile([128, n_mo, s], bf16, tag="QT")
                KT2_sb = bpool.tile([128, s], bf16, tag="KT2")
                VT_sb = bpool.tile([128, s], bf16, tag="VT")
                V_sb = bpool.tile([128, n_skt, 128], bf16, tag="V")
                tiles[("QT", b)] = QT_sb
                tiles[("KT2", b)] = KT2_sb
                tiles[("V", b)] = V_sb
                # ones column / zero pad for PV lhsT
                nc.gpsimd.memset(V_sb[:, :, 64:128], 0.0)
                nc.gpsimd.memset(V_sb[:, :, 64:65], 1.0)
                for m in (2, 0, 1):  # K/V first so attention can start early
                    for n in range(n_nt):
                        nsl = slice(n * QTS, (n + 1) * QTS)
                        ps = psw.tile([128, QTS], f32, tag="w")
                        for kt in range(n_dkt):
                            nc.tensor.matmul(
                                ps[:],
                                wqkv_sb[:, kt, m * 128 : (m + 1) * 128],
                                xT_sb[:, kt, nsl],
                                start=(kt == 0),
                                stop=(kt == n_dkt - 1),
                            )
                        if m < 2:
                            # two Q heads stacked: rope all 128 rows
                            q_raw = wpool.tile([128, QTS], bf16, tag="qraw")
                            nc.scalar.activation(q_raw[:], ps[:], Copy)
                            t1 = wpool.tile([128, QTS], bf16, tag="t1")
                            t2 = wpool.tile([128, QTS], bf16, tag="t2")
                            nc.vector.tensor_tensor(
                                t1[:], q_raw[:], cos_sb[:, nsl], mult_op
                            )
                            # swap(q_raw) via cross-base copies, then * sin
                            qsw = wpool.tile([128, QTS], bf16, tag="qsw")
                            for r0, r1 in ((0, 32), (32, 0), (64, 96), (96, 64)):
                                nc.vector.tensor_copy(
                                    qsw[r0 : r0 + 32, :], q_raw[r1 : r1 + 32, :]
                                )
                            nc.vector.tensor_tensor(
                                t2[:], qsw[:], sin_sb[:, nsl], mult_op
                            )
                            nc.vector.tensor_tensor(
                                QT_sb[:, m, nsl], t1[:], t2[:], add_op
                            )
                        else:
                            # rows 0:64 = K^T (rope), rows 64:128 = V^T (copy)
                            q_raw = wpool.tile([128, QTS], bf16, tag="qraw")
                            nc.scalar.activation(q_raw[0:64, :], ps[0:64, :], Copy)
                            t1 = wpool.tile([128, QTS], bf16, tag="t1")
                            t2 = wpool.tile([128, QTS], bf16, tag="t2")
                            nc.vector.tensor_tensor(
                                t1[0:64, :], q_raw[0:64, :], cos_sb[0:64, nsl], mult_op
                            )
                            qsw = wpool.tile([128, QTS], bf16, tag="qsw")
                            for r0, r1 in ((0, 32), (32, 0)):
                                nc.vector.tensor_copy(
                                    qsw[r0 : r0 + 32, :], q_raw[r1 : r1 + 32, :]
                                )
                            nc.vector.tensor_tensor(
                                t2[0:64, :], qsw[0:64, :], sin_sb[0:64, nsl], mult_op
                            )
                            nc.vector.tensor_tensor(
                                KT2_sb[0:64, nsl], t1[0:64, :], t2[0:64, :], add_op
                            )
                            # duplicate K^T into partitions 64:128 (row-group packing)
                            nc.vector.tensor_copy(
                                KT2_sb[64:128, nsl], KT2_sb[0:64, nsl]
                            )
                            # V^T: plain cast copy into partitions 64:128
                            nc.scalar.activation(
                                VT_sb[64:128, nsl], ps[64:128, :], Copy
                            )
                    if m == 2:
                        # V^T -> V (token-major) via DMA transpose
                        mark(f"b{b}_vtr")
                        for kt in range(n_skt):
                            nc.sync.dma_start_transpose(
                                V_sb[:, kt, 0:64],
                                VT_sb[64:128, kt * KTS : (kt + 1) * KTS],
                            )
                        mark(f"b{b}_proj2")

            def attn(b):
                mark(f"b{b}_attn")
                QT_sb = tiles[("QT", b)]
                KT2_sb = tiles[("KT2", b)]
                V_sb = tiles[("V", b)]
                OT_sb = bpool.tile([128, n_mo, s], bf16, tag="OT")
                tiles[("OT", b)] = OT_sb
                pending = []

                def normalize(hb2, m2, qsl2, ops2, rt2):
                    # recip already issued (DVE); broadcast + scale into OT
                    bps = psw.tile([128, QTS], f32, tag="w")
                    nc.tensor.matmul(
                        bps[0:64, :], ones_sb[:], rt2[:], start=True, stop=True
                    )
                    bsb = wpool.tile([64, QTS], f32, tag="bsb")
                    nc.any.tensor_copy(bsb[:], bps[0:64, :])
                    nc.vector.tensor_tensor(
                        OT_sb[hb2 : hb2 + 64, m2, qsl2],
                        ops2[0:64, :],
                        bsb[:],
                        mult_op,
                    )

                for qt in range(n_qt):
                    for h in range(HQ):
                        hb = (h % 2) * 64
                        qh = QT_sb[hb : hb + 64, h // 2, :]
                        kt2 = KT2_sb[hb : hb + 64, :]
                        qsl = slice(qt * QTS, (qt + 1) * QTS)
                        n_kt = (qt + 1) * (QTS // KTS)  # k tiles needed
                        ops = psops.tile([128, QTS], f32, tag="ops")
                        for g in range(0, n_kt, GRP):
                            kts = list(range(g, min(g + GRP, n_kt)))
                            sc = pssc.tile([128, GRP * QTS], f32, tag="sc")
                            e = epool.tile([128, GRP * QTS], bf16, tag="e")
                            for j, kt in enumerate(kts):
                                nc.tensor.matmul(
                                    sc[:, j * QTS : (j + 1) * QTS],
                                    kt2[:, kt * KTS : (kt + 1) * KTS],
                                    qh[:, qsl],
                                    start=True,
                                    stop=True,
                                )
                            if g == 0 and pending:
                                # normalize the previous q-tile now; its recip
                                # had time to finish, so PE doesn't stall
                                normalize(*pending.pop())
                            nc.scalar.activation(
                                e[:, 0 : len(kts) * QTS], sc[:, 0 : len(kts) * QTS], Exp
                            )
                            for j, kt in enumerate(kts):
                                o = kt * KTS - qt * QTS
                                if o >= 0:  # diagonal tile: 0/1 mask after exp
                                    nc.vector.tensor_tensor(
                                        e[:, j * QTS : (j + 1) * QTS],
                                        e[:, j * QTS : (j + 1) * QTS],
                                        tri01_sb[:, o // KTS, :],
                                        mult_op,
                                    )
                                nc.tensor.matmul(
                                    ops[:],
                                    V_sb[:, kt, :],
                                    e[:, j * QTS : (j + 1) * QTS],
                                    start=(kt == 0),
                                    stop=(kt == n_kt - 1),
                                )
                        rt = wpool.tile([1, QTS], f32, tag="rt")
                        nc.vector.reciprocal(rt[:], ops[64:65, :])
                        pending.append((hb, h // 2, qsl, ops, rt))
                    if qt > 0:
                        wo_block(b, qt - 1)
                if pending:
                    normalize(*pending.pop())
                wo_block(b, n_qt - 1)

            def wo_block(b, qt):
                OT_sb = tiles[("OT", b)]
                for mt in range(4 * qt, 4 * qt + 4):
                    msl = slice(mt * 128, (mt + 1) * 128)
                    osb = opool.tile([128, d], bf16, tag="osb")
                    for n in range(d // QTS):
                        nsl = slice(n * QTS, (n + 1) * QTS)
                        pool = psw if n % 2 == 0 else pssc
                        ps = pool.tile([128, QTS], f32, tag="w" if n % 2 == 0 else "sc")
                        for kt in range(n_mo):
                            nc.tensor.matmul(
                                ps[:],
                                OT_sb[:, kt, msl],
                                wo_sb[:, kt, nsl],
                                start=(kt == 0),
                                stop=(kt == n_mo - 1),
                            )
                        nc.any.tensor_copy(osb[:, nsl], ps[:])
                    nc.sync.dma_start(
                        part_d[b * s + mt * 128 : b * s + (mt + 1) * 128, :],
                        osb[:],
                    )

            load_x(0)
            proj(0)
            load_x(1)  # b1 input load overlaps b0 attention (SP order)
            attn(0)
            proj(1)
            attn(1)
    mark("end")
    nc.compile()
    return nc


# ---------------- host-side sharding ----------------

_PERM = np.concatenate([np.arange(0, HD, 2), np.arange(1, HD, 2)])  # evens, odds


def make_core_inputs(x, freqs_cos, freqs_sin, wq, wk, wv, wo, s=S, d=D):
    """Build per-core input maps (list of dicts, one per core)."""
    b = x.shape[0]
    xT = np.ascontiguousarray(np.transpose(x, (0, 2, 1))).astype(BF16)  # [B, D, S]

    cosT = np.ascontiguousarray(freqs_cos.T)  # [32, S]
    sinT = np.ascontiguousarray(freqs_sin.T)
    cosb = np.tile(np.concatenate([cosT, cosT], axis=0), (2, 1)).astype(BF16)  # [128,S]
    sinb = np.tile(
        np.concatenate([-sinT, sinT], axis=0), (2, 1)
    ).astype(BF16)

    p = np.arange(128)[:, None]
    f = np.arange(128)[None, :]
    trimask = np.where(f >= p, 0.0, -1e9).astype(np.float32)
    f5 = np.arange(QTS)[None, :]
    tri01 = np.stack(
        [np.where(f5 >= o + p, 1.0, 0.0) for o in (0, 128, 256, 384)], axis=1
    ).astype(BF16)  # [128, 4, 512]

    scale = 1.0 / math.sqrt(HD)
    in_maps = []
    for c in range(N_CORES):
        wq_c = np.concatenate(
            [
                wq[:, (4 * c + h) * HD : (4 * c + h + 1) * HD][:, _PERM]
                for h in range(HQ)
            ],
            axis=1,
        ) * scale
        wk_c = wk[:, c * HD : (c + 1) * HD][:, _PERM]
        wv_c = wv[:, c * HD : (c + 1) * HD]
        wqkv = np.concatenate([wq_c, wk_c, wv_c], axis=1).astype(BF16)  # [D, 384]
        wo_c = np.ascontiguousarray(
            wo[4 * c * HD : (4 * c + HQ) * HD, :]
        ).astype(BF16)  # [256, D] — O is in original d-order (V unpermuted)
        in_maps.append(
            {
                "xT": xT,
                "wqkv": wqkv,
                "wo_s": wo_c,
                "cosb": cosb,
                "sinb": sinb,
                "trimask": trimask,
                "tri01": tri01,
            }
        )
    return in_maps


_NC_CACHE = {}


def kernel(x, freqs_cos, freqs_sin, wq, wk, wv, wo):
    from concourse.bass_utils import run_bass_kernel_spmd

    x = np.asarray(x, np.float32)
    freqs_cos = np.asarray(freqs_cos, np.float32)
    freqs_sin = np.asarray(freqs_sin, np.float32)
    wq = np.asarray(wq, np.float32)
    wk = np.asarray(wk, np.float32)
    wv = np.asarray(wv, np.float32)
    wo = np.asarray(wo, np.float32)

    if "nc" not in _NC_CACHE:
        _NC_CACHE["nc"] = build_program()
    nc = _NC_CACHE["nc"]

    in_maps = make_core_inputs(x, freqs_cos, freqs_sin, wq, wk, wv, wo)
    res = run_bass_kernel_spmd(nc, in_maps, list(range(N_CORES)))
    acc = np.zeros((B * S, D), np.float32)
    for r in res.results:
        acc += np.asarray(r["part"], np.float32)
    return acc.reshape(B, S, D).astype(BF16)
